# revision 13
# baseline (speedup 1.0000x reference)
"""Trainium2 Bass kernel for nn_Loss_197568496105 (chamfer-based loss_fn).

Strategy:
 - Data-parallel over batch: core b handles point cloud b (B=8, 8 cores).
 - The heavy work (pairwise-distance min-reductions: pts<->recon_model
   2048x2048 and pts<->kpt 2048x128, both directions) runs on device.
   Squared distances are produced directly by the TensorEngine via an
   augmented matmul:  d2[i,j] = (-2*x_i).y_j + |x_i|^2 * 1 + 1 * |y_j|^2,
   i.e. K=5 contraction rows [-2x, |x|^2, 1] x [y, 1, |y|^2].
   VectorE then does a free-dim min-reduce per 128-row strip (min of d2;
   sqrt is applied on the gathered scalars on host since sqrt is monotonic).
 - The tiny loss components (pose / nocs / diversity / delta) are O(B*K)
   glue computed on host in numpy, as is the final cross-core mean.
"""

import numpy as np

_B = 8
_NCORES = 8
_N = 2048  # pts / recon_model points per batch
_K = 128   # keypoints per batch
_NSTRIP = _N // 128  # 16
_NBANK = _N // 512  # 4 sub-mins per 2048-wide strip
# cols: A (pts vs recon) 64 | C (recon vs pts) 64 | B (pts vs kpt) 16 | D (kpt vs pts) 4
_NCOLS = 2 * _NSTRIP * _NBANK + _NSTRIP + _NBANK  # 148
_AUGW = 4 * _N + 2 * _K  # packed width: lp|rr|lr|rp|lk|rk

TH = 0.1
SL1_BETA = 0.1
EPS = 1e-8

_nc_cache = None


def _build_bass():
    import concourse.bass as bass
    import concourse.mybir as mybir
    import concourse.tile as tile

    f32 = mybir.dt.float32
    X = mybir.AxisListType.X
    MIN = mybir.AluOpType.min

    # The stock kernel-tail emits ONE SP Drain carrying a wait per
    # outstanding semaphore (PE + DVE + one per DMA queue = 4 here); walrus
    # codegen rejects >limit sync-waits per instruction ("Too many sync wait
    # commands"). Split the global clock across several Drains, <=2 waits each.
    import re as _re

    from concourse.vector_clock import ScopedClock, VectorClock

    def _split_drain(self, tick_clock, wait_clock):
        vals = [int(x) for x in _re.findall(r"\d+", repr(tick_clock.global_clock))]
        nz = [(i, v) for i, v in enumerate(vals) if v > 0]
        groups = [nz[i : i + 1] for i in range(0, len(nz), 1)] or [[]]
        for grp in groups:
            c = VectorClock()
            for i, v in grp:
                c.require_at_least(i, v)
            d = self.nc.sync.drain()
            wait_clock.add_sem_waits(d.ins, ScopedClock({None: c}))
        self.nc.all_engine_barrier()
        popped = self.nc._tile_sem_poison_stack.pop()
        assert popped is self._sem_poison
        self.nc.clear_and_free_semaphores(list(self.sems.allocated().values()))
        self.nc.all_engine_barrier()

    tile.TileContext._drain_and_barrier = _split_drain

    nc = bass.Bass()

    aug = nc.dram_tensor("aug", [5, _AUGW], f32, kind="ExternalInput")
    out = nc.dram_tensor("mins", [128, _NCOLS], f32, kind="ExternalOutput")

    with tile.TileContext(nc) as tc:
        with (
            tc.tile_pool(name="sb", bufs=1) as sb,
            tc.tile_pool(name="res", bufs=1) as res,
            tc.tile_pool(name="ps", bufs=8, space="PSUM") as ps,
        ):
            sb_aug = sb.tile([5, _AUGW], f32, tag="aug")
            nc.gpsimd.dma_start(out=sb_aug[:], in_=aug[:])
            sb_lp = sb_aug[:, 0 * _N : 1 * _N]
            sb_rr = sb_aug[:, 1 * _N : 2 * _N]
            sb_lr = sb_aug[:, 2 * _N : 3 * _N]
            sb_rp = sb_aug[:, 3 * _N : 4 * _N]
            sb_lk = sb_aug[:, 4 * _N : 4 * _N + _K]
            sb_rk = sb_aug[:, 4 * _N + _K : 4 * _N + 2 * _K]

            mins = res.tile([128, _NCOLS], f32, tag="mins")

            def bank_pass(lhs_tile, rhs_tile, t, u, out_col, n_rhs):
                """One [128, n_rhs<=512] d2 tile: 1 matmul + 1 min-reduce
                into mins[:, out_col]. Single PE writer + single DVE reader
                per PSUM tile keeps one sync-wait per Matmult (S3_LW limit)."""
                pt = ps.tile([128, 512], f32, tag="ps")
                nc.tensor.matmul(
                    pt[:, 0:n_rhs],
                    lhs_tile[:, t * 128 : (t + 1) * 128],
                    rhs_tile[:, u * 512 : u * 512 + n_rhs],
                    start=True,
                    stop=True,
                )
                nc.vector.tensor_reduce(
                    out=mins[:, out_col : out_col + 1],
                    in_=pt[:, 0:n_rhs],
                    axis=X,
                    op=MIN,
                )

            for t in range(_NSTRIP):
                for u in range(_NBANK):
                    # pts strip t vs recon bank u (loss_recon dir 1)
                    bank_pass(sb_lp, sb_rr, t, u, _NBANK * t + u, 512)
                    # recon strip t vs pts bank u (loss_recon dir 2)
                    bank_pass(
                        sb_lr, sb_rp, t, u,
                        _NSTRIP * _NBANK + _NBANK * t + u, 512,
                    )
                # pts strip t vs kpts (loss_cd axis=2)
                bank_pass(sb_lp, sb_rk, t, 0, 2 * _NSTRIP * _NBANK + t, _K)
            for u in range(_NBANK):
                # kpts vs pts bank u (loss_cd axis=1)
                bank_pass(
                    sb_lk, sb_rp, 0, u,
                    2 * _NSTRIP * _NBANK + _NSTRIP + u, 512,
                )

            nc.gpsimd.dma_start(out=out[:], in_=mins[:])

    return nc


def _get_nc():
    global _nc_cache
    if _nc_cache is None:
        _nc_cache = _build_bass()
    return _nc_cache


def _aug_stationary(x):
    """x: (n,3) float -> [5,n] rows [-2x^T, |x|^2, 1]."""
    x64 = x.astype(np.float64)
    a = np.empty((5, x.shape[0]), np.float32)
    a[0:3] = -2.0 * x64.T
    a[3] = (x64 * x64).sum(1)
    a[4] = 1.0
    return a


def _aug_moving(x):
    """x: (n,3) float -> [5,n] rows [x^T, 1, |x|^2]."""
    x64 = x.astype(np.float64)
    a = np.empty((5, x.shape[0]), np.float32)
    a[0:3] = x64.T
    a[3] = 1.0
    a[4] = (x64 * x64).sum(1)
    return a


def _make_in_maps(pts, recon_model, pred_kpt_3d):
    in_maps = []
    for b in range(_B):
        p = pts[b]
        r = recon_model[b]
        k = pred_kpt_3d[b]
        a = np.empty((5, _AUGW), np.float32)
        a[:, 0 * _N : 1 * _N] = _aug_stationary(p)
        a[:, 1 * _N : 2 * _N] = _aug_moving(r)
        a[:, 2 * _N : 3 * _N] = _aug_stationary(r)
        a[:, 3 * _N : 4 * _N] = _aug_moving(p)
        a[:, 4 * _N : 4 * _N + _K] = _aug_stationary(k)
        a[:, 4 * _N + _K : 4 * _N + 2 * _K] = _aug_moving(k)
        in_maps.append({"aug": a})
    return in_maps


def _host_small_losses(pts, recon_delta, pred_kpt_3d, pred_kpt_nocs,
                       pred_rotation, pred_translation, pred_size,
                       rotation_label, translation_label, size_label):
    f8 = lambda v: v.astype(np.float64)
    b = pts.shape[0]

    loss_pose = (
        np.mean(np.linalg.norm(f8(pred_rotation) - f8(rotation_label), axis=1))
        + np.mean(np.linalg.norm(f8(pred_translation) - f8(translation_label), axis=1))
        + np.mean(np.linalg.norm(f8(pred_size) - f8(size_label), axis=1))
    )

    scale = np.linalg.norm(f8(size_label), axis=1).reshape(b, 1, 1) + EPS
    kpt_nocs_gt = np.einsum(
        "bki,bij->bkj",
        (f8(pred_kpt_3d) - f8(translation_label)[:, None, :]) / scale,
        f8(rotation_label),
    )
    diff = np.abs(f8(pred_kpt_nocs) - kpt_nocs_gt)
    sl1 = np.where(diff > SL1_BETA, diff - SL1_BETA / 2.0, diff * diff / (2.0 * SL1_BETA))
    loss_nocs = np.mean(np.sum(sl1, axis=2))

    kpt = f8(pred_kpt_3d)
    kk = kpt.shape[1]
    eye = np.eye(kk, dtype=bool)[None]
    d2 = ((kpt[:, :, None, :] - kpt[:, None, :, :]) ** 2).sum(-1)
    d2 = np.where(eye, 1.0, d2)
    dm = np.where(eye, np.inf, np.sqrt(d2))
    loss_diversity = np.mean(np.minimum(dm, TH))

    loss_delta = np.mean(np.linalg.norm(f8(recon_delta), axis=2))

    return loss_pose + loss_nocs + loss_diversity + loss_delta


def kernel(pts, recon_delta, pred_kpt_3d, recon_model, pred_kpt_nocs,
           pred_rotation, pred_translation, pred_size,
           rotation_label, translation_label, size_label):
    from concourse.bass_utils import run_bass_kernel_spmd

    nc = _get_nc()

    in_maps = _make_in_maps(pts, recon_model, pred_kpt_3d)

    res = run_bass_kernel_spmd(nc, in_maps, core_ids=list(range(_NCORES)))

    nsb = _NSTRIP * _NBANK
    total = 0.0
    for b in range(_B):
        m = res.results[b]["mins"].astype(np.float64)
        # A/C: [128, 16 strips, 4 banks] -> min over banks
        rmin_pr = np.maximum(m[:, 0:nsb].reshape(128, _NSTRIP, _NBANK).min(2), 0.0)
        cmin_pr = np.maximum(
            m[:, nsb : 2 * nsb].reshape(128, _NSTRIP, _NBANK).min(2), 0.0
        )
        rmin_pk = np.maximum(m[:, 2 * nsb : 2 * nsb + _NSTRIP], 0.0)
        cmin_pk = np.maximum(
            m[:, 2 * nsb + _NSTRIP : 2 * nsb + _NSTRIP + _NBANK].min(1), 0.0
        )
        loss_recon = 0.5 * (np.sqrt(rmin_pr).mean() + np.sqrt(cmin_pr).mean())
        loss_cd = 0.5 * (np.sqrt(cmin_pk).mean() + np.sqrt(rmin_pk).mean())
        total += loss_recon + loss_cd

    total /= _B
    total += _host_small_losses(
        pts, recon_delta, pred_kpt_3d, pred_kpt_nocs,
        pred_rotation, pred_translation, pred_size,
        rotation_label, translation_label, size_label,
    )
    return np.float32(total)
